# revision 1
# baseline (speedup 1.0000x reference)
import sys

sys.path.insert(0, "/opt/trn_rl_repo")

import numpy as np
import ml_dtypes

from concourse import bacc
import concourse.mybir as mybir
from concourse.tile import TileContext
from concourse import bass_utils

_SPARSE = (48, 64, 96, 128, 192, 256, 384, 512, 768, 1024, 1536)
OFFSETS = tuple(range(33)) + _SPARSE
B, H, N, HD = 4, 16, 4096, 64
NEAR = [0, 1, 2]
# far block-distances: two swapped-half pair blocks + one singleton quadrant
PAIRS = [(3, 4), (6, 8)]  # (bdA, bdB), bdB > bdA
SINGLE = 12
NT = N // 128
NBH = 8
NCORES = 8
VW = 66  # v columns: 64 hd + 1 ones + 1 pad
SW = 384 + 128 * len(PAIRS) + 64  # 704 score cols
BF16 = ml_dtypes.bfloat16

# Act does exact exp on [0, ACT_W); DVE fast-exp on [ACT_W, w) (far cols only)
ACT_W = 448
# Schraudolph fast-exp: int16(s*A + B) bit pattern read as bf16 ~= exp(s)
FE_A = float(128.0 / np.log(2.0))
FE_B = float(127 * 128 - 5.60 + 0.5)  # +0.5: float->int cast truncates

_nc_cache = None


def _build_bass():
    nc = bacc.Bacc("TRN2", target_bir_lowering=False)
    qkt = nc.dram_tensor("qkt", [NBH, 64, 2 * N], mybir.dt.bfloat16, kind="ExternalInput")
    # va | vp0 | vp1 | em concatenated along free dim (single DMA per bh)
    VALL = NT * VW + 2 * (NT * 2 * VW) + SW
    vall = nc.dram_tensor("vall", [NBH, 128, VALL], mybir.dt.bfloat16, kind="ExternalInput")
    out = nc.dram_tensor(
        "out", [NBH, NT // 8, 128, 8 * VW], mybir.dt.bfloat16, kind="ExternalOutput"
    )

    with TileContext(nc) as tc:
        with (
            tc.tile_pool(name="io", bufs=2) as io_pool,
            tc.tile_pool(name="spsum", bufs=2, space="PSUM") as spool,
            tc.tile_pool(name="opsum", bufs=2, space="PSUM") as opool,
            tc.tile_pool(name="work", bufs=3) as wpool,
            tc.tile_pool(name="osb", bufs=3) as opool_sb,
        ):
            for bh in range(NBH):
                qk_sb = io_pool.tile([64, 2 * N], mybir.dt.bfloat16, tag="qkt")
                qt_sb = qk_sb[:, :N]
                kt_sb = qk_sb[:, N:]
                VALL = NT * VW + 2 * (NT * 2 * VW) + SW
                vall_sb = io_pool.tile([128, VALL], mybir.dt.bfloat16, tag="vall")
                va_sb = vall_sb[:, : NT * VW]
                vp_sb = [
                    vall_sb[:, NT * VW + i * NT * 2 * VW : NT * VW + (i + 1) * NT * 2 * VW]
                    for i in range(len(PAIRS))
                ]
                em_sb = vall_sb[:, NT * VW + 2 * NT * 2 * VW :]
                nc.sync.dma_start(qk_sb[:, :], qkt[bh])
                nc.sync.dma_start(vall_sb[:, :], vall[bh])
                op = None
                for t in range(NT):
                    nn = sum(1 for bd in NEAR if t - bd >= 0)
                    npair = sum(1 for (bdA, bdB) in PAIRS if t - bdA >= 0)
                    has12 = t - SINGLE >= 0
                    w = 128 * nn + 128 * npair + (64 if has12 else 0)
                    sp = spool.tile([128, SW], mybir.dt.float32, tag="sp")
                    for i in range(nn):
                        bd = NEAR[i]
                        nc.tensor.matmul(
                            sp[:, i * 128 : (i + 1) * 128],
                            kt_sb[:, (t - bd) * 128 : (t - bd + 1) * 128],
                            qt_sb[:, t * 128 : (t + 1) * 128],
                            start=True,
                            stop=True,
                        )
                    # far pair blocks (swapped-half layout), cols c0..c0+127:
                    #  q h0 cols: parts 0..63 <- keys[t-bdB, 0:64], 64..127 <- keys[t-bdA, 0:64]
                    #  q h1 cols: parts 0..63 <- keys[t-bdA, 64:128], 64..127 <- keys[t-bdB, 64:128]
                    for pi, (bdA, bdB) in enumerate(PAIRS):
                        if t - bdA < 0:
                            continue
                        c0 = 384 + pi * 128
                        ta = t - bdA
                        tb = max(t - bdB, 0)  # clamped garbage at fallback tiles
                        q0 = qt_sb[:, t * 128 : t * 128 + 64]
                        q1 = qt_sb[:, t * 128 + 64 : t * 128 + 128]
                        nc.tensor.matmul(
                            sp[0:64, c0 : c0 + 64],
                            kt_sb[:, tb * 128 : tb * 128 + 64],
                            q0, start=True, stop=True,
                            tile_position=(0, 0),
                        )
                        nc.tensor.matmul(
                            sp[64:128, c0 : c0 + 64],
                            kt_sb[:, ta * 128 : ta * 128 + 64],
                            q0, start=True, stop=True,
                            tile_position=(0, 64),
                            skip_group_check=True,
                        )
                        nc.tensor.matmul(
                            sp[0:64, c0 + 64 : c0 + 128],
                            kt_sb[:, ta * 128 + 64 : (ta + 1) * 128],
                            q1, start=True, stop=True,
                            tile_position=(0, 0),
                        )
                        nc.tensor.matmul(
                            sp[64:128, c0 + 64 : c0 + 128],
                            kt_sb[:, tb * 128 + 64 : (tb + 1) * 128],
                            q1, start=True, stop=True,
                            tile_position=(0, 64),
                            skip_group_check=True,
                        )
                    if has12:
                        c0 = 384 + len(PAIRS) * 128
                        t12 = t - SINGLE
                        for hh in (0, 1):
                            nc.tensor.matmul(
                                sp[64 * hh : 64 * hh + 64, c0 : c0 + 64],
                                kt_sb[:, t12 * 128 + 64 * hh : t12 * 128 + 64 * hh + 64],
                                qt_sb[:, t * 128 + 64 * hh : t * 128 + 64 * hh + 64],
                                start=True, stop=True,
                                tile_position=(0, 64 * hh),
                                skip_group_check=(hh == 1),
                            )
                    ex = wpool.tile([128, SW], mybir.dt.bfloat16, tag="ex")
                    a = min(w, ACT_W)
                    nc.scalar.activation(
                        ex[:, :a], sp[:, :a], mybir.ActivationFunctionType.Exp
                    )
                    if w > a:
                        nc.vector.tensor_scalar(
                            ex[:, a:w].bitcast(mybir.dt.int16),
                            sp[:, a:w],
                            FE_A,
                            FE_B,
                            mybir.AluOpType.mult,
                            mybir.AluOpType.add,
                        )
                    # mask+bias multiply on gpsimd (SBUF-only engine, off DVE)
                    exm = wpool.tile([128, SW], mybir.dt.bfloat16, tag="exm")
                    nc.gpsimd.tensor_tensor(
                        exm[:, :w], ex[:, :w], em_sb[:, :w], mybir.AluOpType.mult
                    )
                    if t % 2 == 0:
                        op = opool.tile([128, 2 * VW], mybir.dt.float32, tag="op")
                    osl = op[:, (t % 2) * VW : (t % 2 + 1) * VW]
                    # (out_ap, lhsT, rhs, tile_position)
                    # order: full-region near blocks first (start) and last
                    # (stop) so every psum byte's accumulation group closes.
                    mm = []
                    for i in range(1, nn):
                        bd = NEAR[i]
                        mm.append((
                            osl,
                            exm[:, i * 128 : (i + 1) * 128],
                            va_sb[:, (t - bd) * VW : (t - bd + 1) * VW],
                            None,
                        ))
                    for pi, (bdA, bdB) in enumerate(PAIRS):
                        if t - bdA < 0:
                            continue
                        c0 = 384 + pi * 128
                        vps = vp_sb[pi]
                        ta = t - bdA
                        mm.append((
                            osl[0:64, :],
                            exm[:, c0 : c0 + 64],
                            vps[:, ta * 2 * VW : ta * 2 * VW + VW],
                            (0, 0),
                        ))
                        mm.append((
                            osl[64:128, :],
                            exm[:, c0 + 64 : c0 + 128],
                            vps[:, ta * 2 * VW + VW : (ta + 1) * 2 * VW],
                            (0, 64),
                        ))
                    if has12:
                        c0 = 384 + len(PAIRS) * 128
                        t12 = t - SINGLE
                        for hh in (0, 1):
                            mm.append((
                                osl[64 * hh : 64 * hh + 64, :],
                                exm[64 * hh : 64 * hh + 64, c0 : c0 + 64],
                                va_sb[64 * hh : 64 * hh + 64, t12 * VW : (t12 + 1) * VW],
                                (64 * hh, 64 * hh),
                            ))
                    mm.append((
                        osl,
                        exm[:, 0:128],
                        va_sb[:, t * VW : (t + 1) * VW],
                        None,
                    ))
                    for i, (oap, lh, rh, tp) in enumerate(mm):
                        kw = {}
                        if tp is not None:
                            kw["tile_position"] = tp
                            kw["skip_group_check"] = True
                        nc.tensor.matmul(
                            oap, lh, rh,
                            start=(i == 0),
                            stop=(i == len(mm) - 1),
                            **kw,
                        )
                    if t % 8 == 1:
                        osb = opool_sb.tile([128, 8 * VW], mybir.dt.bfloat16, tag="osb")
                    if t % 2 == 1:
                        g2 = (t % 8) // 2
                        nc.vector.tensor_copy(
                            osb[:, g2 * 2 * VW : (g2 + 1) * 2 * VW], op[:, :]
                        )
                    if t % 8 == 7:
                        nc.sync.dma_start(out[bh, t // 8], osb[:, :])
    nc.compile()
    return nc


def _host_prep(q, k, v, pos_bias):
    qf = q.reshape(B * H, N, HD)
    kf = k.reshape(B * H, N, HD)
    vf = v.reshape(B * H, N, HD)
    sc = 1.0 / np.sqrt(HD)

    lut = np.full(1537, -1, np.int64)
    for i, d in enumerate(OFFSETS):
        lut[d] = i
    row = np.arange(128)[:, None]  # partition = key row n
    col = np.arange(128)[None, :]  # free = query m
    em_heads = np.zeros((H, 128, SW), np.float32)
    for bdi, bd in enumerate(NEAR):
        dd = 128 * bd + col - row
        ok = (dd >= 0) & (dd <= 1536)
        idx = np.where(ok, lut[np.clip(dd, 0, 1536)], -1)
        valid = idx >= 0
        for h in range(H):
            vals = np.where(valid, np.exp(pos_bias[np.clip(idx, 0, 43), h]), 0.0)
            em_heads[h, :, bdi * 128 : (bdi + 1) * 128] = vals
    d64 = np.eye(64, dtype=np.float32)
    for pi, (bdA, bdB) in enumerate(PAIRS):
        iA, iB = lut[128 * bdA], lut[128 * bdB]
        c0 = 384 + pi * 128
        for h in range(H):
            eA, eB = np.exp(pos_bias[iA, h]), np.exp(pos_bias[iB, h])
            blk = np.zeros((128, 128), np.float32)
            blk[0:64, 0:64] = d64 * eB      # q h0, keys t-bdB rows 0..63
            blk[64:128, 0:64] = d64 * eA    # q h0, keys t-bdA rows 0..63
            blk[0:64, 64:128] = d64 * eA    # q h1, keys t-bdA rows 64..127
            blk[64:128, 64:128] = d64 * eB  # q h1, keys t-bdB rows 64..127
            em_heads[h, :, c0 : c0 + 128] = blk
    i12 = lut[128 * SINGLE]
    c0 = 384 + len(PAIRS) * 128
    for h in range(H):
        e12 = np.exp(pos_bias[i12, h])
        em_heads[h, 0:64, c0 : c0 + 64] = d64 * e12
        em_heads[h, 64:128, c0 : c0 + 64] = d64 * e12

    in_maps = []
    for c in range(NCORES):
        bhs = list(range(c * NBH, (c + 1) * NBH))
        qkt = np.concatenate(
            [np.stack([(qf[j].T * sc) for j in bhs]),
             np.stack([kf[j].T for j in bhs])], axis=2).astype(BF16)
        vaug = np.concatenate(
            [vf[bhs], np.ones((NBH, N, 1), np.float32), np.zeros((NBH, N, 1), np.float32)],
            axis=2,
        )  # [NBH, N, VW]
        va = np.zeros((NBH, 128, NT * VW), np.float32)
        for jj in range(NBH):
            vt = vaug[jj].reshape(NT, 128, VW)
            va[jj] = vt.transpose(1, 0, 2).reshape(128, NT * VW)
        # pair-block swapped-half v buffers, indexed by tau = t - bdB:
        #  h0 group: parts 0..63 = v tile tau rows 0..63;  64..127 = tile tau+dlt rows 0..63
        #  h1 group: parts 0..63 = tile tau+dlt rows 64..127; 64..127 = tile tau rows 64..127
        vpb = np.zeros((NBH, len(PAIRS), 128, NT, 2, VW), np.float32)
        for pi, (bdA, bdB) in enumerate(PAIRS):
            dlt = bdB - bdA
            for jj in range(NBH):
                vx = vaug[jj]
                for ta in range(NT):
                    lo = 128 * (ta - dlt)  # tile of bdB (may be invalid)
                    hi = 128 * ta          # tile of bdA
                    vpb[jj, pi, 64:128, ta, 0] = vx[hi : hi + 64]
                    vpb[jj, pi, 0:64, ta, 1] = vx[hi + 64 : hi + 128]
                    if ta - dlt >= 0:
                        vpb[jj, pi, 0:64, ta, 0] = vx[lo : lo + 64]
                        vpb[jj, pi, 64:128, ta, 1] = vx[lo + 64 : lo + 128]
        em = np.stack([em_heads[j % H] for j in bhs])
        vall = np.concatenate(
            [va, vpb.reshape(NBH, len(PAIRS), 128, NT * 2 * VW).transpose(0, 2, 1, 3)
                 .reshape(NBH, 128, -1), em],
            axis=2,
        )
        in_maps.append({
            "qkt": np.ascontiguousarray(qkt),
            "vall": np.ascontiguousarray(vall.astype(BF16)),
        })
    return in_maps


def kernel(q, k, v, pos_bias):
    global _nc_cache
    if _nc_cache is None:
        _nc_cache = _build_bass()
    nc = _nc_cache
    in_maps = _host_prep(
        np.asarray(q, np.float32), np.asarray(k, np.float32),
        np.asarray(v, np.float32), np.asarray(pos_bias, np.float32),
    )
    res = bass_utils.run_bass_kernel_spmd(nc, in_maps, core_ids=list(range(NCORES)))
    outs = [r["out"] for r in res.results]  # each [NBH, NT//8, 128, 8*VW] bf16
    raw = np.concatenate(outs, axis=0).astype(np.float32)
    raw = raw.reshape(B * H, NT // 8, 128, 8, VW)
    raw = raw.transpose(0, 1, 3, 2, 4).reshape(B * H, N, VW)
    full = raw[:, :, :HD] / raw[:, :, HD : HD + 1]
    return full.reshape(B, H, N, HD).astype(np.float32)

